# revision 32
# baseline (speedup 1.0000x reference)
"""Trainium2 Bass/Tile kernel for nn_DualStreamSDENet (8-core data parallel).

Self-contained: hardcodes shapes/sharding; accepts FULL inputs, returns the
FULL output tuple (mean [B], log_sigma [B], dir_logit [B,2]) matching
reference.py. Batch 16384 is split 2048/core across 8 NeuronCores; weights
are packed host-side into one f32 and one bf16 constant tensor (lhsT layout,
LN affine folded into the head weights, attention softmax-of-2 rewritten as
sigmoid of the per-head score difference).

Per-core pipeline (feature-major [feature, batch] on chip):
- x_seq T-sum is done INLINE by chained SWDGE accumulate-DMAs (CCE add,
  4 links/block); small DVE combine + PE transpose produce x_mean
  feature-major. x_seq never lands raw in SBUF.
- The 25-step SDE loop keeps W@h resident in PSUM via cross-step matmul
  accumulation in 4 independent batch-quarter chains: ACT computes
  dt*relu(psA+b) in bf16, DVE adds the noise term (levy cast to bf16
  during DMA, scaled by the broadcast noise coefficient), and PE folds
  u = dt*relu + noise back into psA (drift) and psS (state sum) with
  bf16 matmuls. h_25 = h_0 + psS.
- Attention/LN/head run as two 1024-column macro-chunks of bf16 matmuls
  with per-512 PSUM drains; k/v differences are projected from hL-hF
  (biases cancel), q stays in PSUM, 1/std = exp(-0.5*ln(var+eps)) on ACT.
"""
import os
import sys

for _p in ("/opt/trn_rl_repo", "/root/.axon_site/_ro/trn_rl_repo"):
    if os.path.isdir(_p) and _p not in sys.path:
        sys.path.insert(0, _p)

import numpy as np

import concourse.bass as bass
import concourse.bacc as bacc
import concourse.tile as tile
from concourse import mybir
from concourse.bass_utils import run_bass_kernel_spmd

F32 = mybir.dt.float32
BF16 = mybir.dt.bfloat16
ALU = mybir.AluOpType
ACTF = mybir.ActivationFunctionType

N_CORES = 8
B, T, D, H, HEADS, AUX, MID, GATE_MID = 16384, 64, 64, 128, 4, 10, 100, 16
BL = B // N_CORES          # 2048 per core
DEPTH = 25
DT = 1.0 / DEPTH
SIGMA_MIN, SIGMA_MAX = 0.1, 1.5
ALPHA = 1.2
KAPPA = DT ** (1.0 / ALPHA)
LN_EPS = 1e-5
DH = H // HEADS            # 32
NBLK = BL // 128           # 16 batch blocks
NC512 = BL // 512          # 4 chunks
XLINKS = 4                 # accumulate-DMA chain depth per x block

# ---------------------------------------------------------------------------
# Packed-constant layout: one [128, NCOLS] f32 tensor.
# entry: name -> (rows, cols, col_offset)
_LAYOUT = {}
_NCOLS = 0


def _add(name, rows, cols):
    global _NCOLS
    _LAYOUT[name] = (rows, cols, _NCOLS)
    _NCOLS += cols


_add("drwT", 128, 128)      # dr_w.T
_add("I128", 128, 128)      # identity (psS accum + PE transpose)
_add("inwT_q", 128, 128)
_add("inwT_k", 128, 128)
_add("inwT_v", 128, 128)
_add("outwT", 128, 128)
_add("trwT_L", 128, 128)    # (tr_w * g_rep)[:, :128].T
_add("trwT_F", 128, 128)
_add("dswT", 64, 128)
_add("fswT_x", 64, 128)
_add("fswT_a", 10, 128)
_add("dfw1T", 128, 100)
_add("dfw2T", 100, 1)
_add("gw1T", 10, 16)
_add("gw2T", 16, 1)
_add("headWT", 128, 4)
_add("blkA", 128, 4)        # per-head 1/sqrt(DH) reduce
_add("blkB", 4, 128)        # per-head broadcast
_add("ones1", 1, 128)       # rank-1 partition broadcast
_add("uL", 1, 128)
_add("uF", 1, 128)
_add("onesLN", 128, 1)      # 1/H column for LN mean
_add("dsb", 128, 1)
_add("drbdt", 128, 1)
_add("dfb1", 100, 1)
_add("dfb2", 1, 1)
_add("gb1", 16, 1)
_add("gb2", 1, 1)
_add("fsb", 128, 1)
_add("inb_q", 128, 1)
_add("inb_k", 128, 1)
_add("inb_v", 128, 1)
_add("outb", 128, 1)
_add("trbE", 128, 1)
_add("headb", 4, 1)
_add("eps", 1, 1)
_add("bqA", 128, 4)

WCOLS = _NCOLS

_LAYOUT16 = {}
_NCOLS16 = 0


def _add16(name, rows, cols):
    global _NCOLS16
    _LAYOUT16[name] = (rows, cols, _NCOLS16)
    _NCOLS16 += cols


for _n in ("drwT", "I128", "inwT_q", "inwT_k", "inwT_v", "outwT", "trwT_L",
           "trwT_F", "fswT_x", "fswT_a", "headWT", "blkA", "blkB", "ones1",
           "uL", "uF", "onesLN", "bqA"):
    _add16(_n, _LAYOUT[_n][0], _LAYOUT[_n][1])
WCOLS16 = _NCOLS16


def _pack_consts(i):
    """Build the [128, WCOLS] packed constant array from the weight dict."""
    w = np.zeros((128, WCOLS), dtype=np.float32)

    def put(name, arr):
        rows, cols, off = _LAYOUT[name]
        a = np.asarray(arr, dtype=np.float32).reshape(rows, cols)
        w[:rows, off:off + cols] = a

    g_rep = np.concatenate([i["ln_g"], i["ln_g"]])          # [256]
    b_rep = np.concatenate([i["ln_b"], i["ln_b"]])          # [256]
    trw_eff = np.asarray(i["tr_w"], np.float32) * g_rep[None, :]   # [128,256]

    put("drwT", np.asarray(i["dr_w"]).T)
    put("I128", np.eye(128, dtype=np.float32))
    inw = np.asarray(i["in_w"])                              # [384,128]
    put("inwT_q", inw[0:128].T)
    put("inwT_k", inw[128:256].T)
    put("inwT_v", inw[256:384].T)
    put("outwT", np.asarray(i["out_w"]).T)
    put("trwT_L", trw_eff[:, 0:128].T)
    put("trwT_F", trw_eff[:, 128:256].T)
    put("dswT", np.asarray(i["ds_w"]).T)
    fsw = np.asarray(i["fs_w"])                              # [128, 74]
    put("fswT_x", fsw[:, 0:D].T)
    put("fswT_a", fsw[:, D:D + AUX].T)
    put("dfw1T", np.asarray(i["df_w1"]).T)
    put("dfw2T", np.asarray(i["df_w2"]).T)
    put("gw1T", np.asarray(i["g_w1"]).T)
    put("gw2T", np.asarray(i["g_w2"]).T)
    headW = np.concatenate(
        [np.asarray(i["hm_w"]), np.asarray(i["hs_w"]), np.asarray(i["hd_w"])],
        axis=0)                                              # [4,128]
    put("headWT", headW.T)
    blkA = np.zeros((128, 4), np.float32)
    blkB = np.zeros((4, 128), np.float32)
    for hd in range(HEADS):
        blkA[hd * DH:(hd + 1) * DH, hd] = 1.0 / np.sqrt(DH)
        blkB[hd, hd * DH:(hd + 1) * DH] = 1.0
    put("blkA", blkA)
    put("blkB", blkB)
    put("ones1", np.ones((1, 128), np.float32))
    put("uL", trw_eff[:, 0:128].sum(axis=1)[None, :])
    put("uF", trw_eff[:, 128:256].sum(axis=1)[None, :])
    put("onesLN", np.full((128, 1), 1.0 / H, np.float32))
    put("dsb", i["ds_b"])
    put("drbdt", np.asarray(i["dr_b"]) * DT)
    put("dfb1", i["df_b1"])
    put("dfb2", i["df_b2"])
    put("gb1", i["g_b1"])
    put("gb2", i["g_b2"])
    put("fsb", i["fs_b"])
    inb = np.asarray(i["in_b"])
    put("inb_q", inb[0:128])
    put("inb_k", inb[128:256])
    put("inb_v", inb[256:384])
    put("outb", i["out_b"])
    trbE = np.asarray(i["tr_b"]) + np.asarray(i["tr_w"]) @ b_rep
    put("trbE", trbE)
    put("headb", np.concatenate(
        [np.asarray(i["hm_b"]), np.asarray(i["hs_b"]), np.asarray(i["hd_b"])]))
    put("eps", np.array([[LN_EPS]], np.float32))
    bqA = np.zeros((128, 4), np.float32)
    for hd in range(HEADS):
        bqA[hd * DH:(hd + 1) * DH, hd] = inb[0:128][hd * DH:(hd + 1) * DH] \
            / np.sqrt(DH)
    put("bqA", bqA)
    return w


# ---------------------------------------------------------------------------
def build_nc():
    nc = bacc.Bacc("TRN2", target_bir_lowering=False, debug=False,
                   num_swdge_queues=4)
    xseq = nc.declare_dram_parameter("xseq", [BL, T, D], F32, isOutput=False)
    levy = nc.declare_dram_parameter("levy", [DEPTH, H, BL], F32,
                                     isOutput=False)
    xlast = nc.declare_dram_parameter("xlast", [D, BL], F32, isOutput=False)
    auxf = nc.declare_dram_parameter("auxf", [AUX, BL], F32, isOutput=False)
    wc = nc.declare_dram_parameter("wconst", [128, WCOLS], F32,
                                   isOutput=False)
    wc16 = nc.declare_dram_parameter("wconst16", [128, WCOLS16], BF16,
                                     isOutput=False)
    out4 = nc.declare_dram_parameter("out4", [4, BL], F32, isOutput=True)

    with tile.TileContext(nc) as tc:
        _body(nc, tc, xseq, levy, xlast, auxf, wc, wc16, out4)
    nc.finalize()
    return nc


def _body(nc, tc, xseq, levy, xlast, auxf, wc, wc16, out4):
    import contextlib
    ctx = contextlib.ExitStack()
    with ctx:
        persist = ctx.enter_context(tc.tile_pool(name="persist", bufs=1))
        etap = ctx.enter_context(tc.tile_pool(name="eta", bufs=3))
        enp = ctx.enter_context(tc.tile_pool(name="en", bufs=2))
        r2p = ctx.enter_context(tc.tile_pool(name="r2", bufs=4))
        accp = ctx.enter_context(tc.tile_pool(name="acc", bufs=8))
        tcp = ctx.enter_context(tc.tile_pool(name="tc", bufs=9))
        rowp = ctx.enter_context(tc.tile_pool(name="rows", bufs=8))
        psp = ctx.enter_context(
            tc.tile_pool(name="ps", bufs=8, space=bass.MemorySpace.PSUM))

        # ---- persistent tiles -------------------------------------------
        wt = persist.tile([128, WCOLS], F32, tag="wt")
        nc.sync.dma_start(wt[:], wc[:])
        wt16 = persist.tile([128, WCOLS16], BF16, tag="wt16")
        nc.sync.dma_start(wt16[:], wc16[:])

        def W16(name):
            rows, cols, off = _LAYOUT16[name]
            return wt16[0:rows, off:off + cols]

        drwT16 = W16("drwT")
        I12816 = W16("I128")

        def W(name):
            rows, cols, off = _LAYOUT[name]
            return wt[0:rows, off:off + cols]

        xlast_t = persist.tile([D, BL], F32, tag="xlast")
        aux_t = persist.tile([AUX, BL], F32, tag="aux")
        nc.sync.dma_start(xlast_t[:], xlast[:])
        nc.sync.dma_start(aux_t[:], auxf[:])

        h0 = persist.tile([128, BL], F32, tag="h0")
        ncb = persist.tile([128, BL], BF16, tag="ncb")
        hL16s = [persist.tile([128, 1024], BF16, tag=f"hL16{k}",
                              name=f"hL16{k}") for k in range(2)]
        hF16s = [persist.tile([128, 1024], BF16, tag=f"hF16{k}",
                              name=f"hF16{k}") for k in range(2)]
        xmeans = [persist.tile([D, 1024], BF16, tag=f"xmean{k}",
                               name=f"xmean{k}") for k in range(2)]
        xsum = persist.tile([128, NBLK * D], F32, tag="xsum")
        out4_t = persist.tile([4, BL], F32, tag="o4")

        C = 512

        # ---- pre-loop (per 512 chunk): h0, gate, micro, noise coef ------
        for c in range(NC512):
            cs = slice(c * C, (c + 1) * C)
            ps = psp.tile([128, C], F32, tag="ps")
            nc.tensor.matmul(ps[:], W("dswT"), xlast_t[:, cs])
            nc.scalar.activation(h0[:, cs], ps[:], ACTF.Identity,
                                 bias=W("dsb"))
            psg = psp.tile([GATE_MID, C], F32, tag="ps")
            nc.tensor.matmul(psg[:], W("gw1T"), aux_t[:, cs])
            g1c = tcp.tile([GATE_MID, C], F32, tag="tc", bufs=4)
            nc.scalar.activation(g1c[:], psg[:], ACTF.Relu, bias=W("gb1"))
            psg2 = psp.tile([1, C], F32, tag="ps")
            nc.tensor.matmul(psg2[:], W("gw2T"), g1c[:])
            g2c = rowp.tile([1, C], F32, tag="rows", bufs=3)
            nc.scalar.activation(g2c[:], psg2[:], ACTF.Sigmoid, bias=W("gb2"))
            psd = psp.tile([MID, C], F32, tag="ps")
            nc.tensor.matmul(psd[:], W("dfw1T"), h0[:, cs])
            dm1c = tcp.tile([MID, C], F32, tag="tc", bufs=4)
            nc.scalar.activation(dm1c[:], psd[:], ACTF.Relu, bias=W("dfb1"))
            psd2 = psp.tile([1, C], F32, tag="ps")
            nc.tensor.matmul(psd2[:], W("dfw2T"), dm1c[:])
            dm2c = rowp.tile([1, C], F32, tag="rows", bufs=3)
            nc.scalar.activation(dm2c[:], psd2[:], ACTF.Sigmoid,
                                 bias=W("dfb2"))
            # ncrow = ((1.4*g2 + 0.1) * kappa) * dm2
            nc.vector.tensor_scalar(g2c[:], g2c[:],
                                    (SIGMA_MAX - SIGMA_MIN) * KAPPA,
                                    SIGMA_MIN * KAPPA, ALU.mult, ALU.add)
            ncrc = rowp.tile([1, C], F32, tag="rows", bufs=3)
            nc.vector.tensor_tensor(ncrc[:], g2c[:], dm2c[:], ALU.mult)
            psb = psp.tile([128, C], F32, tag="ps")
            nc.tensor.matmul(psb[:], W("ones1"), ncrc[:])
            nc.scalar.copy(ncb[:, cs], psb[:])

        # ---- x_seq streaming T-sum via accumulate-DMA chains ------------
        ncols = (T * D) // XLINKS          # 1024 free cols per link
        for blk in range(NBLK):
            acc = accp.tile([128, ncols], F32, tag="acc")
            src = xseq[blk * 128:(blk + 1) * 128].rearrange("b t d -> b (t d)")
            for j in range(XLINKS):
                sl = src[:, j * ncols:(j + 1) * ncols]
                if j == 0:
                    nc.sync.dma_start(acc[:], sl)
                else:
                    nc.gpsimd.dma_start(acc[:], sl, accum_op=ALU.add)
            nc.vector.tensor_reduce(
                xsum[:, blk * D:(blk + 1) * D],
                acc[:].rearrange("p (t d) -> p d t", d=D),
                axis=mybir.AxisListType.X, op=ALU.add)

        # ---- SDE loop: PSUM-resident state ------------------------------
        # psA_h = W~.T @ h_t (halves), psS_h accumulates (dt*relu + en)
        psA = [psp.tile([128, C], F32, tag="ps", name=f"psA{k}")
               for k in range(4)]
        psS = [psp.tile([128, C], F32, tag="ps", name=f"psS{k}")
               for k in range(4)]
        for q in range(4):
            cs = slice(q * C, (q + 1) * C)
            nc.tensor.matmul(psA[q][:], W("drwT"), h0[:, cs],
                             start=True, stop=False)

        for t in range(DEPTH):
            eta = etap.tile([128, BL], BF16, tag="eta")
            nc.gpsimd.dma_start(eta[:], levy[t])
            en = enp.tile([128, BL], BF16, tag="en")
            nc.vector.tensor_tensor(en[:], eta[:], ncb[:], ALU.mult)
            last = t == DEPTH - 1
            for q in range(4):
                qs = slice(q * C, (q + 1) * C)
                r2 = r2p.tile([128, C], BF16, tag="r2")
                nc.scalar.activation(r2[:], psA[q][:], ACTF.Relu,
                                     bias=W("drbdt"), scale=DT)
                u = r2p.tile([128, C], BF16, tag="u")
                nc.vector.tensor_tensor(u[:], r2[:], en[:, qs], ALU.add)
                if not last:
                    nc.tensor.matmul(psA[q][:], drwT16, u[:],
                                     start=False, stop=(t == DEPTH - 2))
                nc.tensor.matmul(psS[q][:], I12816, u[:],
                                 start=(t == 0), stop=last)

        # h_25 = h_0 + psS
        for q in range(4):
            qs = slice(q * C, (q + 1) * C)
            ls = slice((q % 2) * C, (q % 2 + 1) * C)
            nc.vector.scalar_tensor_tensor(hL16s[q // 2][:, ls], psS[q][:],
                                           1.0, h0[:, qs], ALU.mult, ALU.add)

        aux16 = persist.tile([AUX, BL], BF16, tag="aux16")
        nc.scalar.copy(aux16[:], aux_t[:])

        # ---- xmean: PE transpose of per-block sums, scale 1/T -----------
        for blk in range(NBLK):
            pst = psp.tile([D, 128], F32, tag="ps")
            nc.tensor.transpose(pst[:], xsum[:, blk * D:(blk + 1) * D],
                                W("I128"))
            lb = (blk % 8) * 128
            nc.scalar.mul(xmeans[blk // 8][:, lb:lb + 128], pst[:], 1.0 / T)

        # ---- tail: 2 macro-chunks of 1024 cols; mms per 512 ---------------
        MW = 1024
        for mh in range(2):
            g0 = mh * MW                   # global col offset
            gsl = slice(g0, g0 + MW)
            hLm, hFm, xmm = hL16s[mh], hF16s[mh], xmeans[mh]

            def mm2(w16name, dst_f, src, actf, bias, extra=None,
                    wname=None, src_off=g0, dst2_f=None):
                """dst[j-slice] = actf(W.T @ src[...] + bias), per 512"""
                for j in range(2):
                    ja = slice(j * C, (j + 1) * C)
                    jg = slice(src_off + j * C, src_off + (j + 1) * C)
                    pw = psp.tile([128, C], F32, tag="ps",
                                  name=f"pw{nc.next_id()}")
                    lhs = W16(w16name) if w16name else W(wname)
                    nc.tensor.matmul(pw[:], lhs, src[:, jg],
                                     start=True, stop=(extra is None))
                    if extra is not None:
                        nc.tensor.matmul(pw[:], W16(extra[0]),
                                         extra[1][:, jg], start=False,
                                         stop=True)
                    nc.scalar.activation(dst_f(ja), pw[:], actf, bias=bias)
                    if dst2_f is not None:
                        nc.scalar.activation(dst2_f(ja), pw[:], actf,
                                             bias=bias)

            # H_feat (xmean local, aux16 global)
            for j in range(2):
                ja = slice(j * C, (j + 1) * C)
                jg = slice(g0 + j * C, g0 + (j + 1) * C)
                pw = psp.tile([128, C], F32, tag="ps")
                nc.tensor.matmul(pw[:], W16("fswT_x"), xmm[:, ja],
                                 start=True, stop=False)
                nc.tensor.matmul(pw[:], W16("fswT_a"), aux16[:, jg],
                                 start=False, stop=True)
                nc.scalar.activation(hFm[:, ja], pw[:], ACTF.Relu,
                                     bias=W("fsb"))

            def proj(wname, bname, src, src_off=None):
                dst = tcp.tile([128, MW], BF16, tag="tcw")
                mm2(wname, lambda ja: dst[:, ja], src, ACTF.Identity,
                    W(bname) if bname else 0.0,
                    src_off=g0 if src_off is None else src_off)
                return dst

            hd16 = tcp.tile([128, MW], BF16, tag="tcw")
            nc.vector.tensor_sub(hd16[:], hLm[:], hFm[:])
            # kd = W_k @ (hL-hF)  (biases cancel); vd = W_v @ (hL-hF)
            kd = proj("inwT_k", None, hd16, src_off=0)
            vd = proj("inwT_v", None, hd16, src_off=0)
            vF = proj("inwT_v", "inb_v", hFm, src_off=0)

            resid, rstds, shrs = [], [], []
            prods, prows_l, ms_l = [], [], []
            # prod_s = (W_q @ h_s) * kd, with q left in PSUM
            for h_s in (hLm, hFm):
                prod = tcp.tile([128, MW], BF16, tag="tcw")
                for j in range(2):
                    ja = slice(j * C, (j + 1) * C)
                    jg = slice(g0 + j * C, g0 + (j + 1) * C)
                    pq = psp.tile([128, C], F32, tag="ps")
                    nc.tensor.matmul(pq[:], W16("inwT_q"), h_s[:, ja])
                    nc.vector.tensor_tensor(prod[:, ja], pq[:], kd[:, ja],
                                            ALU.mult)
                prods.append(prod)
            for prod in prods:
                prow = rowp.tile([HEADS, MW], BF16, tag="prow", bufs=2)
                for j in range(2):
                    ja = slice(j * C, (j + 1) * C)
                    pr = psp.tile([HEADS, C], F32, tag="ps")
                    nc.tensor.matmul(pr[:], W16("blkA"), prod[:, ja],
                                     start=True, stop=False)
                    nc.tensor.matmul(pr[:], W16("bqA"), kd[:, ja],
                                     start=False, stop=True)
                    nc.scalar.activation(prow[:, ja], pr[:], ACTF.Sigmoid)
                prows_l.append(prow)
            for prow in prows_l:
                m_s = tcp.tile([128, MW], BF16, tag="tcw")
                for j in range(2):
                    ja = slice(j * C, (j + 1) * C)
                    pb = psp.tile([128, C], F32, tag="ps")
                    nc.tensor.matmul(pb[:], W16("blkB"), prow[:, ja])
                    nc.vector.tensor_tensor(m_s[:, ja], pb[:], vd[:, ja],
                                            ALU.mult)
                ms_l.append(m_s)
            for six, m_s in enumerate(ms_l):
                seq_s = hLm if six == 0 else hFm
                ctx = tcp.tile([128, MW], BF16, tag="tcw")
                nc.vector.tensor_add(ctx[:], vF[:], m_s[:])
                r_s = tcp.tile([128, MW], BF16, tag="tcw")
                for j in range(2):
                    ja = slice(j * C, (j + 1) * C)
                    jg = slice(g0 + j * C, g0 + (j + 1) * C)
                    po = psp.tile([128, C], F32, tag="ps")
                    nc.tensor.matmul(po[:], W16("outwT"), ctx[:, ja])
                    nc.vector.scalar_tensor_tensor(r_s[:, ja], po[:],
                                                   W("outb"), seq_s[:, ja],
                                                   ALU.add, ALU.add)
                resid.append(r_s)

            murs, vAs, vBs = [], [], []
            for r_s in resid:
                sq = tcp.tile([128, MW], BF16, tag="tcw")
                nc.scalar.activation(sq[:], r_s[:], ACTF.Square)
                mur = rowp.tile([1, MW], F32, tag="rowsw", bufs=6)
                vA = rowp.tile([1, MW], F32, tag="rowsw", bufs=6)
                vB = rowp.tile([1, MW], F32, tag="rowsw", bufs=6)
                for j in range(2):
                    ja = slice(j * C, (j + 1) * C)
                    psmu = psp.tile([1, C], F32, tag="ps")
                    nc.tensor.matmul(psmu[:], W16("onesLN"), r_s[:, ja])
                    nc.scalar.copy(mur[:, ja], psmu[:])
                    nc.vector.tensor_tensor(vB[:, ja], mur[:, ja],
                                            mur[:, ja], ALU.mult)
                    psex = psp.tile([1, C], F32, tag="ps")
                    nc.tensor.matmul(psex[:], W16("onesLN"), sq[:, ja])
                    # vA = Ex2 - mu^2, Ex2 read straight from PSUM
                    nc.vector.scalar_tensor_tensor(
                        vA[:, ja], vB[:, ja], -1.0, psex[:],
                        ALU.mult, ALU.add)
                murs.append(mur)
                vAs.append(vA)
                vBs.append(vB)
            for vA, vB in zip(vAs, vBs):
                nc.scalar.activation(vB[:], vA[:], ACTF.Ln, bias=W("eps"))
            for vA, vB in zip(vAs, vBs):
                nc.scalar.activation(vA[:], vB[:], ACTF.Exp, scale=-0.5)
            for mur, vA, vB in zip(murs, vAs, vBs):
                nc.vector.scalar_tensor_tensor(vB[:], mur[:], -1.0, vA[:],
                                               ALU.mult, ALU.mult)
                rstds.append(vA)
                shrs.append(vB)

            nms = []
            for r_s, rstd in zip(resid, rstds):
                nm = tcp.tile([128, MW], BF16, tag="tcw")
                for j in range(2):
                    ja = slice(j * C, (j + 1) * C)
                    pbc = psp.tile([128, C], F32, tag="ps")
                    nc.tensor.matmul(pbc[:], W("ones1"), rstd[:, ja])
                    nc.vector.tensor_tensor(nm[:, ja], r_s[:, ja], pbc[:],
                                            ALU.mult)
                nms.append(nm)

            hhx = tcp.tile([128, MW], BF16, tag="tcw")
            for j in range(2):
                ja = slice(j * C, (j + 1) * C)
                ph = psp.tile([128, C], F32, tag="ps")
                nc.tensor.matmul(ph[:], W16("trwT_L"), nms[0][:, ja],
                                 start=True, stop=False)
                nc.tensor.matmul(ph[:], W16("trwT_F"), nms[1][:, ja],
                                 start=False, stop=False)
                nc.tensor.matmul(ph[:], W("uL"), shrs[0][:, ja],
                                 start=False, stop=False)
                nc.tensor.matmul(ph[:], W("uF"), shrs[1][:, ja],
                                 start=False, stop=True)
                nc.scalar.activation(hhx[:, ja], ph[:], ACTF.Relu,
                                     bias=W("trbE"))

            for j in range(2):
                ja = slice(j * C, (j + 1) * C)
                jg = slice(g0 + j * C, g0 + (j + 1) * C)
                pq = psp.tile([4, C], F32, tag="ps")
                nc.tensor.matmul(pq[:], W16("headWT"), hhx[:, ja])
                nc.scalar.activation(out4_t[:, jg], pq[:], ACTF.Identity,
                                     bias=W("headb"))

        nc.sync.dma_start(out4[:], out4_t[:])


# ---------------------------------------------------------------------------
_NC_CACHE = None
LAST_RESULTS = None


def kernel(**inputs):
    global _NC_CACHE, LAST_RESULTS
    i = {k: np.asarray(v) for k, v in inputs.items()}
    x_seq = np.asarray(i["x_seq"], np.float32)
    aux = np.asarray(i["aux_feat"], np.float32)
    levy = np.asarray(i["levy_noise"], np.float32)

    wconst = _pack_consts(i)
    import ml_dtypes
    wc16 = np.zeros((128, WCOLS16), ml_dtypes.bfloat16)
    for _n, (_r, _c, _o16) in _LAYOUT16.items():
        _, _, _o = _LAYOUT[_n]
        wc16[:_r, _o16:_o16 + _c] = wconst[:_r, _o:_o + _c].astype(
            ml_dtypes.bfloat16)

    in_maps = []
    for c in range(N_CORES):
        sl = slice(c * BL, (c + 1) * BL)
        in_maps.append({
            "xseq": np.ascontiguousarray(x_seq[sl]),
            "levy": np.ascontiguousarray(levy[:, sl, :].transpose(0, 2, 1)),
            "xlast": np.ascontiguousarray(x_seq[sl, -1, :].T),
            "auxf": np.ascontiguousarray(aux[sl].T),
            "wconst": wconst,
            "wconst16": wc16,
        })

    if _NC_CACHE is None:
        _NC_CACHE = build_nc()
    res = run_bass_kernel_spmd(_NC_CACHE, in_maps, core_ids=list(range(N_CORES)))
    LAST_RESULTS = res

    mean = np.empty((B,), np.float32)
    log_sigma = np.empty((B,), np.float32)
    dir_logit = np.empty((B, 2), np.float32)
    for c in range(N_CORES):
        o = res.results[c]["out4"]           # [4, BL]
        sl = slice(c * BL, (c + 1) * BL)
        mean[sl] = o[0]
        log_sigma[sl] = o[1]
        dir_logit[sl, 0] = o[2]
        dir_logit[sl, 1] = o[3]
    return mean, log_sigma, dir_logit


# revision 33
# speedup vs baseline: 1.0628x; 1.0628x over previous
"""Trainium2 Bass/Tile kernel for nn_DualStreamSDENet (8-core data parallel).

Self-contained: hardcodes shapes/sharding; accepts FULL inputs, returns the
FULL output tuple (mean [B], log_sigma [B], dir_logit [B,2]) matching
reference.py. Batch 16384 is split 2048/core across 8 NeuronCores; weights
are packed host-side into one f32 and one bf16 constant tensor (lhsT layout,
LN affine folded into the head weights, attention softmax-of-2 rewritten as
sigmoid of the per-head score difference).

Per-core pipeline (feature-major [feature, batch] on chip):
- x_seq T-sum is done INLINE by chained SWDGE accumulate-DMAs (CCE add,
  4 links/block); small DVE combine + PE transpose produce x_mean
  feature-major. x_seq never lands raw in SBUF.
- The 25-step SDE loop keeps W@h resident in PSUM via cross-step matmul
  accumulation in 4 independent batch-quarter chains: ACT computes
  dt*relu(psA+b) in bf16, DVE adds the noise term (levy cast to bf16
  during DMA, scaled by the broadcast noise coefficient), and PE folds
  u = dt*relu + noise back into psA (drift) and psS (state sum) with
  bf16 matmuls. h_25 = h_0 + psS.
- Attention/LN/head run as two 1024-column macro-chunks of bf16 matmuls
  with per-512 PSUM drains; k/v differences are projected from hL-hF
  (biases cancel), q stays in PSUM, 1/std = exp(-0.5*ln(var+eps)) on ACT.
"""
import os
import sys

for _p in ("/opt/trn_rl_repo", "/root/.axon_site/_ro/trn_rl_repo"):
    if os.path.isdir(_p) and _p not in sys.path:
        sys.path.insert(0, _p)

import numpy as np

import concourse.bass as bass
import concourse.bacc as bacc
import concourse.tile as tile
from concourse import mybir
from concourse.bass_utils import run_bass_kernel_spmd

F32 = mybir.dt.float32
BF16 = mybir.dt.bfloat16
ALU = mybir.AluOpType
ACTF = mybir.ActivationFunctionType

N_CORES = 8
B, T, D, H, HEADS, AUX, MID, GATE_MID = 16384, 64, 64, 128, 4, 10, 100, 16
BL = B // N_CORES          # 2048 per core
DEPTH = 25
DT = 1.0 / DEPTH
SIGMA_MIN, SIGMA_MAX = 0.1, 1.5
ALPHA = 1.2
KAPPA = DT ** (1.0 / ALPHA)
LN_EPS = 1e-5
DH = H // HEADS            # 32
NBLK = BL // 128           # 16 batch blocks
NC512 = BL // 512          # 4 chunks
XLINKS = 2                 # accumulate-DMA chain depth per x block

# ---------------------------------------------------------------------------
# Packed-constant layout: one [128, NCOLS] f32 tensor.
# entry: name -> (rows, cols, col_offset)
_LAYOUT = {}
_NCOLS = 0


def _add(name, rows, cols):
    global _NCOLS
    _LAYOUT[name] = (rows, cols, _NCOLS)
    _NCOLS += cols


_add("drwT", 128, 128)      # dr_w.T
_add("I128", 128, 128)      # identity (psS accum + PE transpose)
_add("inwT_q", 128, 128)
_add("inwT_k", 128, 128)
_add("inwT_v", 128, 128)
_add("outwT", 128, 128)
_add("trwT_L", 128, 128)    # (tr_w * g_rep)[:, :128].T
_add("trwT_F", 128, 128)
_add("dswT", 64, 128)
_add("fswT_x", 64, 128)
_add("fswT_a", 10, 128)
_add("dfw1T", 128, 100)
_add("dfw2T", 100, 1)
_add("gw1T", 10, 16)
_add("gw2T", 16, 1)
_add("headWT", 128, 4)
_add("blkA", 128, 4)        # per-head 1/sqrt(DH) reduce
_add("blkB", 4, 128)        # per-head broadcast
_add("ones1", 1, 128)       # rank-1 partition broadcast
_add("uL", 1, 128)
_add("uF", 1, 128)
_add("onesLN", 128, 1)      # 1/H column for LN mean
_add("dsb", 128, 1)
_add("drbdt", 128, 1)
_add("dfb1", 100, 1)
_add("dfb2", 1, 1)
_add("gb1", 16, 1)
_add("gb2", 1, 1)
_add("fsb", 128, 1)
_add("inb_q", 128, 1)
_add("inb_k", 128, 1)
_add("inb_v", 128, 1)
_add("outb", 128, 1)
_add("trbE", 128, 1)
_add("headb", 4, 1)
_add("eps", 1, 1)
_add("bqA", 128, 4)

WCOLS = _NCOLS

_LAYOUT16 = {}
_NCOLS16 = 0


def _add16(name, rows, cols):
    global _NCOLS16
    _LAYOUT16[name] = (rows, cols, _NCOLS16)
    _NCOLS16 += cols


for _n in ("drwT", "I128", "inwT_q", "inwT_k", "inwT_v", "outwT", "trwT_L",
           "trwT_F", "fswT_x", "fswT_a", "headWT", "blkA", "blkB", "ones1",
           "uL", "uF", "onesLN", "bqA"):
    _add16(_n, _LAYOUT[_n][0], _LAYOUT[_n][1])
WCOLS16 = _NCOLS16


def _pack_consts(i):
    """Build the [128, WCOLS] packed constant array from the weight dict."""
    w = np.zeros((128, WCOLS), dtype=np.float32)

    def put(name, arr):
        rows, cols, off = _LAYOUT[name]
        a = np.asarray(arr, dtype=np.float32).reshape(rows, cols)
        w[:rows, off:off + cols] = a

    g_rep = np.concatenate([i["ln_g"], i["ln_g"]])          # [256]
    b_rep = np.concatenate([i["ln_b"], i["ln_b"]])          # [256]
    trw_eff = np.asarray(i["tr_w"], np.float32) * g_rep[None, :]   # [128,256]

    put("drwT", np.asarray(i["dr_w"]).T)
    put("I128", np.eye(128, dtype=np.float32))
    inw = np.asarray(i["in_w"])                              # [384,128]
    put("inwT_q", inw[0:128].T)
    put("inwT_k", inw[128:256].T)
    put("inwT_v", inw[256:384].T)
    put("outwT", np.asarray(i["out_w"]).T)
    put("trwT_L", trw_eff[:, 0:128].T)
    put("trwT_F", trw_eff[:, 128:256].T)
    put("dswT", np.asarray(i["ds_w"]).T)
    fsw = np.asarray(i["fs_w"])                              # [128, 74]
    put("fswT_x", fsw[:, 0:D].T)
    put("fswT_a", fsw[:, D:D + AUX].T)
    put("dfw1T", np.asarray(i["df_w1"]).T)
    put("dfw2T", np.asarray(i["df_w2"]).T)
    put("gw1T", np.asarray(i["g_w1"]).T)
    put("gw2T", np.asarray(i["g_w2"]).T)
    headW = np.concatenate(
        [np.asarray(i["hm_w"]), np.asarray(i["hs_w"]), np.asarray(i["hd_w"])],
        axis=0)                                              # [4,128]
    put("headWT", headW.T)
    blkA = np.zeros((128, 4), np.float32)
    blkB = np.zeros((4, 128), np.float32)
    for hd in range(HEADS):
        blkA[hd * DH:(hd + 1) * DH, hd] = 1.0 / np.sqrt(DH)
        blkB[hd, hd * DH:(hd + 1) * DH] = 1.0
    put("blkA", blkA)
    put("blkB", blkB)
    put("ones1", np.ones((1, 128), np.float32))
    put("uL", trw_eff[:, 0:128].sum(axis=1)[None, :])
    put("uF", trw_eff[:, 128:256].sum(axis=1)[None, :])
    put("onesLN", np.full((128, 1), 1.0 / H, np.float32))
    put("dsb", i["ds_b"])
    put("drbdt", np.asarray(i["dr_b"]) * DT)
    put("dfb1", i["df_b1"])
    put("dfb2", i["df_b2"])
    put("gb1", i["g_b1"])
    put("gb2", i["g_b2"])
    put("fsb", i["fs_b"])
    inb = np.asarray(i["in_b"])
    put("inb_q", inb[0:128])
    put("inb_k", inb[128:256])
    put("inb_v", inb[256:384])
    put("outb", i["out_b"])
    trbE = np.asarray(i["tr_b"]) + np.asarray(i["tr_w"]) @ b_rep
    put("trbE", trbE)
    put("headb", np.concatenate(
        [np.asarray(i["hm_b"]), np.asarray(i["hs_b"]), np.asarray(i["hd_b"])]))
    put("eps", np.array([[LN_EPS]], np.float32))
    bqA = np.zeros((128, 4), np.float32)
    for hd in range(HEADS):
        bqA[hd * DH:(hd + 1) * DH, hd] = inb[0:128][hd * DH:(hd + 1) * DH] \
            / np.sqrt(DH)
    put("bqA", bqA)
    return w


# ---------------------------------------------------------------------------
def build_nc():
    nc = bacc.Bacc("TRN2", target_bir_lowering=False, debug=False,
                   num_swdge_queues=4)
    xseq = nc.declare_dram_parameter("xseq", [BL, T, D], F32, isOutput=False)
    levy = nc.declare_dram_parameter("levy", [DEPTH, H, BL], F32,
                                     isOutput=False)
    xlast = nc.declare_dram_parameter("xlast", [D, BL], F32, isOutput=False)
    auxf = nc.declare_dram_parameter("auxf", [AUX, BL], F32, isOutput=False)
    wc = nc.declare_dram_parameter("wconst", [128, WCOLS], F32,
                                   isOutput=False)
    wc16 = nc.declare_dram_parameter("wconst16", [128, WCOLS16], BF16,
                                     isOutput=False)
    out4 = nc.declare_dram_parameter("out4", [4, BL], F32, isOutput=True)

    with tile.TileContext(nc) as tc:
        _body(nc, tc, xseq, levy, xlast, auxf, wc, wc16, out4)
    nc.finalize()
    return nc


def _body(nc, tc, xseq, levy, xlast, auxf, wc, wc16, out4):
    import contextlib
    ctx = contextlib.ExitStack()
    with ctx:
        persist = ctx.enter_context(tc.tile_pool(name="persist", bufs=1))
        etap = ctx.enter_context(tc.tile_pool(name="eta", bufs=3))
        enp = ctx.enter_context(tc.tile_pool(name="en", bufs=2))
        r2p = ctx.enter_context(tc.tile_pool(name="r2", bufs=4))
        accp = ctx.enter_context(tc.tile_pool(name="acc", bufs=4))
        tcp = ctx.enter_context(tc.tile_pool(name="tc", bufs=9))
        rowp = ctx.enter_context(tc.tile_pool(name="rows", bufs=8))
        psp = ctx.enter_context(
            tc.tile_pool(name="ps", bufs=8, space=bass.MemorySpace.PSUM))

        # ---- persistent tiles -------------------------------------------
        wt = persist.tile([128, WCOLS], F32, tag="wt")
        nc.sync.dma_start(wt[:], wc[:])
        wt16 = persist.tile([128, WCOLS16], BF16, tag="wt16")
        nc.sync.dma_start(wt16[:], wc16[:])

        def W16(name):
            rows, cols, off = _LAYOUT16[name]
            return wt16[0:rows, off:off + cols]

        drwT16 = W16("drwT")
        I12816 = W16("I128")

        def W(name):
            rows, cols, off = _LAYOUT[name]
            return wt[0:rows, off:off + cols]

        xlast_t = persist.tile([D, BL], F32, tag="xlast")
        aux_t = persist.tile([AUX, BL], F32, tag="aux")
        nc.sync.dma_start(xlast_t[:], xlast[:])
        nc.sync.dma_start(aux_t[:], auxf[:])

        h0 = persist.tile([128, BL], F32, tag="h0")
        ncb = persist.tile([128, BL], BF16, tag="ncb")
        hL16s = [persist.tile([128, 1024], BF16, tag=f"hL16{k}",
                              name=f"hL16{k}") for k in range(2)]
        hF16s = [persist.tile([128, 1024], BF16, tag=f"hF16{k}",
                              name=f"hF16{k}") for k in range(2)]
        xmeans = [persist.tile([D, 1024], BF16, tag=f"xmean{k}",
                               name=f"xmean{k}") for k in range(2)]
        xsum = persist.tile([128, NBLK * D], F32, tag="xsum")
        out4_t = persist.tile([4, BL], F32, tag="o4")

        C = 512

        # ---- pre-loop (per 512 chunk): h0, gate, micro, noise coef ------
        for c in range(NC512):
            cs = slice(c * C, (c + 1) * C)
            ps = psp.tile([128, C], F32, tag="ps")
            nc.tensor.matmul(ps[:], W("dswT"), xlast_t[:, cs])
            nc.scalar.activation(h0[:, cs], ps[:], ACTF.Identity,
                                 bias=W("dsb"))
            psg = psp.tile([GATE_MID, C], F32, tag="ps")
            nc.tensor.matmul(psg[:], W("gw1T"), aux_t[:, cs])
            g1c = tcp.tile([GATE_MID, C], F32, tag="tc", bufs=4)
            nc.scalar.activation(g1c[:], psg[:], ACTF.Relu, bias=W("gb1"))
            psg2 = psp.tile([1, C], F32, tag="ps")
            nc.tensor.matmul(psg2[:], W("gw2T"), g1c[:])
            g2c = rowp.tile([1, C], F32, tag="rows", bufs=3)
            nc.scalar.activation(g2c[:], psg2[:], ACTF.Sigmoid, bias=W("gb2"))
            psd = psp.tile([MID, C], F32, tag="ps")
            nc.tensor.matmul(psd[:], W("dfw1T"), h0[:, cs])
            dm1c = tcp.tile([MID, C], F32, tag="tc", bufs=4)
            nc.scalar.activation(dm1c[:], psd[:], ACTF.Relu, bias=W("dfb1"))
            psd2 = psp.tile([1, C], F32, tag="ps")
            nc.tensor.matmul(psd2[:], W("dfw2T"), dm1c[:])
            dm2c = rowp.tile([1, C], F32, tag="rows", bufs=3)
            nc.scalar.activation(dm2c[:], psd2[:], ACTF.Sigmoid,
                                 bias=W("dfb2"))
            # ncrow = ((1.4*g2 + 0.1) * kappa) * dm2
            nc.vector.tensor_scalar(g2c[:], g2c[:],
                                    (SIGMA_MAX - SIGMA_MIN) * KAPPA,
                                    SIGMA_MIN * KAPPA, ALU.mult, ALU.add)
            ncrc = rowp.tile([1, C], F32, tag="rows", bufs=3)
            nc.vector.tensor_tensor(ncrc[:], g2c[:], dm2c[:], ALU.mult)
            psb = psp.tile([128, C], F32, tag="ps")
            nc.tensor.matmul(psb[:], W("ones1"), ncrc[:])
            nc.scalar.copy(ncb[:, cs], psb[:])

        # ---- x_seq streaming T-sum via accumulate-DMA chains ------------
        ncols = (T * D) // XLINKS          # 1024 free cols per link
        for blk in range(NBLK):
            acc = accp.tile([128, ncols], F32, tag="acc")
            src = xseq[blk * 128:(blk + 1) * 128].rearrange("b t d -> b (t d)")
            for j in range(XLINKS):
                sl = src[:, j * ncols:(j + 1) * ncols]
                if j == 0:
                    nc.sync.dma_start(acc[:], sl)
                else:
                    nc.gpsimd.dma_start(acc[:], sl, accum_op=ALU.add)
            nc.vector.tensor_reduce(
                xsum[:, blk * D:(blk + 1) * D],
                acc[:].rearrange("p (t d) -> p d t", d=D),
                axis=mybir.AxisListType.X, op=ALU.add)

        # ---- SDE loop: PSUM-resident state ------------------------------
        # psA_h = W~.T @ h_t (halves), psS_h accumulates (dt*relu + en)
        psA = [psp.tile([128, C], F32, tag="ps", name=f"psA{k}")
               for k in range(4)]
        psS = [psp.tile([128, C], F32, tag="ps", name=f"psS{k}")
               for k in range(4)]
        for q in range(4):
            cs = slice(q * C, (q + 1) * C)
            nc.tensor.matmul(psA[q][:], W("drwT"), h0[:, cs],
                             start=True, stop=False)

        for t in range(DEPTH):
            eta = etap.tile([128, BL], BF16, tag="eta")
            nc.gpsimd.dma_start(eta[:], levy[t])
            en = enp.tile([128, BL], BF16, tag="en")
            nc.vector.tensor_tensor(en[:], eta[:], ncb[:], ALU.mult)
            last = t == DEPTH - 1
            for q in range(4):
                qs = slice(q * C, (q + 1) * C)
                r2 = r2p.tile([128, C], BF16, tag="r2")
                nc.scalar.activation(r2[:], psA[q][:], ACTF.Relu,
                                     bias=W("drbdt"), scale=DT)
                u = r2p.tile([128, C], BF16, tag="u")
                nc.vector.tensor_tensor(u[:], r2[:], en[:, qs], ALU.add)
                if not last:
                    nc.tensor.matmul(psA[q][:], drwT16, u[:],
                                     start=False, stop=(t == DEPTH - 2))
                nc.tensor.matmul(psS[q][:], I12816, u[:],
                                 start=(t == 0), stop=last)

        # h_25 = h_0 + psS
        for q in range(4):
            qs = slice(q * C, (q + 1) * C)
            ls = slice((q % 2) * C, (q % 2 + 1) * C)
            nc.vector.scalar_tensor_tensor(hL16s[q // 2][:, ls], psS[q][:],
                                           1.0, h0[:, qs], ALU.mult, ALU.add)

        aux16 = persist.tile([AUX, BL], BF16, tag="aux16")
        nc.scalar.copy(aux16[:], aux_t[:])

        # ---- xmean: PE transpose of per-block sums, scale 1/T -----------
        for blk in range(NBLK):
            pst = psp.tile([D, 128], F32, tag="ps")
            nc.tensor.transpose(pst[:], xsum[:, blk * D:(blk + 1) * D],
                                W("I128"))
            lb = (blk % 8) * 128
            nc.scalar.mul(xmeans[blk // 8][:, lb:lb + 128], pst[:], 1.0 / T)

        # ---- tail: 2 macro-chunks of 1024 cols; mms per 512 ---------------
        MW = 1024
        for mh in range(2):
            g0 = mh * MW                   # global col offset
            gsl = slice(g0, g0 + MW)
            hLm, hFm, xmm = hL16s[mh], hF16s[mh], xmeans[mh]

            def mm2(w16name, dst_f, src, actf, bias, extra=None,
                    wname=None, src_off=g0, dst2_f=None):
                """dst[j-slice] = actf(W.T @ src[...] + bias), per 512"""
                for j in range(2):
                    ja = slice(j * C, (j + 1) * C)
                    jg = slice(src_off + j * C, src_off + (j + 1) * C)
                    pw = psp.tile([128, C], F32, tag="ps",
                                  name=f"pw{nc.next_id()}")
                    lhs = W16(w16name) if w16name else W(wname)
                    nc.tensor.matmul(pw[:], lhs, src[:, jg],
                                     start=True, stop=(extra is None))
                    if extra is not None:
                        nc.tensor.matmul(pw[:], W16(extra[0]),
                                         extra[1][:, jg], start=False,
                                         stop=True)
                    nc.scalar.activation(dst_f(ja), pw[:], actf, bias=bias)
                    if dst2_f is not None:
                        nc.scalar.activation(dst2_f(ja), pw[:], actf,
                                             bias=bias)

            # H_feat (xmean local, aux16 global)
            for j in range(2):
                ja = slice(j * C, (j + 1) * C)
                jg = slice(g0 + j * C, g0 + (j + 1) * C)
                pw = psp.tile([128, C], F32, tag="ps")
                nc.tensor.matmul(pw[:], W16("fswT_x"), xmm[:, ja],
                                 start=True, stop=False)
                nc.tensor.matmul(pw[:], W16("fswT_a"), aux16[:, jg],
                                 start=False, stop=True)
                nc.scalar.activation(hFm[:, ja], pw[:], ACTF.Relu,
                                     bias=W("fsb"))

            def proj(wname, bname, src, src_off=None):
                dst = tcp.tile([128, MW], BF16, tag="tcw")
                mm2(wname, lambda ja: dst[:, ja], src, ACTF.Identity,
                    W(bname) if bname else 0.0,
                    src_off=g0 if src_off is None else src_off)
                return dst

            hd16 = tcp.tile([128, MW], BF16, tag="tcw")
            nc.vector.tensor_sub(hd16[:], hLm[:], hFm[:])
            # kd = W_k @ (hL-hF)  (biases cancel); vd = W_v @ (hL-hF)
            kd = proj("inwT_k", None, hd16, src_off=0)
            vd = proj("inwT_v", None, hd16, src_off=0)
            vF = proj("inwT_v", "inb_v", hFm, src_off=0)

            resid, rstds, shrs = [], [], []
            prods, prows_l, ms_l = [], [], []
            # prod_s = (W_q @ h_s) * kd, with q left in PSUM
            for h_s in (hLm, hFm):
                prod = tcp.tile([128, MW], BF16, tag="tcw")
                for j in range(2):
                    ja = slice(j * C, (j + 1) * C)
                    jg = slice(g0 + j * C, g0 + (j + 1) * C)
                    pq = psp.tile([128, C], F32, tag="ps")
                    nc.tensor.matmul(pq[:], W16("inwT_q"), h_s[:, ja])
                    nc.vector.tensor_tensor(prod[:, ja], pq[:], kd[:, ja],
                                            ALU.mult)
                prods.append(prod)
            for prod in prods:
                prow = rowp.tile([HEADS, MW], BF16, tag="prow", bufs=2)
                for j in range(2):
                    ja = slice(j * C, (j + 1) * C)
                    pr = psp.tile([HEADS, C], F32, tag="ps")
                    nc.tensor.matmul(pr[:], W16("blkA"), prod[:, ja],
                                     start=True, stop=False)
                    nc.tensor.matmul(pr[:], W16("bqA"), kd[:, ja],
                                     start=False, stop=True)
                    nc.scalar.activation(prow[:, ja], pr[:], ACTF.Sigmoid)
                prows_l.append(prow)
            for prow in prows_l:
                m_s = tcp.tile([128, MW], BF16, tag="tcw")
                for j in range(2):
                    ja = slice(j * C, (j + 1) * C)
                    pb = psp.tile([128, C], F32, tag="ps")
                    nc.tensor.matmul(pb[:], W16("blkB"), prow[:, ja])
                    nc.vector.tensor_tensor(m_s[:, ja], pb[:], vd[:, ja],
                                            ALU.mult)
                ms_l.append(m_s)
            for six, m_s in enumerate(ms_l):
                seq_s = hLm if six == 0 else hFm
                ctx = tcp.tile([128, MW], BF16, tag="tcw")
                nc.vector.tensor_add(ctx[:], vF[:], m_s[:])
                r_s = tcp.tile([128, MW], BF16, tag="tcw")
                for j in range(2):
                    ja = slice(j * C, (j + 1) * C)
                    jg = slice(g0 + j * C, g0 + (j + 1) * C)
                    po = psp.tile([128, C], F32, tag="ps")
                    nc.tensor.matmul(po[:], W16("outwT"), ctx[:, ja])
                    nc.vector.scalar_tensor_tensor(r_s[:, ja], po[:],
                                                   W("outb"), seq_s[:, ja],
                                                   ALU.add, ALU.add)
                resid.append(r_s)

            murs, vAs, vBs = [], [], []
            for r_s in resid:
                sq = tcp.tile([128, MW], BF16, tag="tcw")
                nc.scalar.activation(sq[:], r_s[:], ACTF.Square)
                mur = rowp.tile([1, MW], F32, tag="rowsw", bufs=6)
                vA = rowp.tile([1, MW], F32, tag="rowsw", bufs=6)
                vB = rowp.tile([1, MW], F32, tag="rowsw", bufs=6)
                for j in range(2):
                    ja = slice(j * C, (j + 1) * C)
                    psmu = psp.tile([1, C], F32, tag="ps")
                    nc.tensor.matmul(psmu[:], W16("onesLN"), r_s[:, ja])
                    nc.scalar.copy(mur[:, ja], psmu[:])
                    nc.vector.tensor_tensor(vB[:, ja], mur[:, ja],
                                            mur[:, ja], ALU.mult)
                    psex = psp.tile([1, C], F32, tag="ps")
                    nc.tensor.matmul(psex[:], W16("onesLN"), sq[:, ja])
                    # vA = Ex2 - mu^2, Ex2 read straight from PSUM
                    nc.vector.scalar_tensor_tensor(
                        vA[:, ja], vB[:, ja], -1.0, psex[:],
                        ALU.mult, ALU.add)
                murs.append(mur)
                vAs.append(vA)
                vBs.append(vB)
            for vA, vB in zip(vAs, vBs):
                nc.scalar.activation(vB[:], vA[:], ACTF.Ln, bias=W("eps"))
            for vA, vB in zip(vAs, vBs):
                nc.scalar.activation(vA[:], vB[:], ACTF.Exp, scale=-0.5)
            for mur, vA, vB in zip(murs, vAs, vBs):
                nc.vector.scalar_tensor_tensor(vB[:], mur[:], -1.0, vA[:],
                                               ALU.mult, ALU.mult)
                rstds.append(vA)
                shrs.append(vB)

            nms = []
            for r_s, rstd in zip(resid, rstds):
                nm = tcp.tile([128, MW], BF16, tag="tcw")
                for j in range(2):
                    ja = slice(j * C, (j + 1) * C)
                    pbc = psp.tile([128, C], F32, tag="ps")
                    nc.tensor.matmul(pbc[:], W("ones1"), rstd[:, ja])
                    nc.vector.tensor_tensor(nm[:, ja], r_s[:, ja], pbc[:],
                                            ALU.mult)
                nms.append(nm)

            hhx = tcp.tile([128, MW], BF16, tag="tcw")
            for j in range(2):
                ja = slice(j * C, (j + 1) * C)
                ph = psp.tile([128, C], F32, tag="ps")
                nc.tensor.matmul(ph[:], W16("trwT_L"), nms[0][:, ja],
                                 start=True, stop=False)
                nc.tensor.matmul(ph[:], W16("trwT_F"), nms[1][:, ja],
                                 start=False, stop=False)
                nc.tensor.matmul(ph[:], W("uL"), shrs[0][:, ja],
                                 start=False, stop=False)
                nc.tensor.matmul(ph[:], W("uF"), shrs[1][:, ja],
                                 start=False, stop=True)
                nc.scalar.activation(hhx[:, ja], ph[:], ACTF.Relu,
                                     bias=W("trbE"))

            for j in range(2):
                ja = slice(j * C, (j + 1) * C)
                jg = slice(g0 + j * C, g0 + (j + 1) * C)
                pq = psp.tile([4, C], F32, tag="ps")
                nc.tensor.matmul(pq[:], W16("headWT"), hhx[:, ja])
                nc.scalar.activation(out4_t[:, jg], pq[:], ACTF.Identity,
                                     bias=W("headb"))

        nc.sync.dma_start(out4[:], out4_t[:])


# ---------------------------------------------------------------------------
_NC_CACHE = None
LAST_RESULTS = None


def kernel(**inputs):
    global _NC_CACHE, LAST_RESULTS
    i = {k: np.asarray(v) for k, v in inputs.items()}
    x_seq = np.asarray(i["x_seq"], np.float32)
    aux = np.asarray(i["aux_feat"], np.float32)
    levy = np.asarray(i["levy_noise"], np.float32)

    wconst = _pack_consts(i)
    import ml_dtypes
    wc16 = np.zeros((128, WCOLS16), ml_dtypes.bfloat16)
    for _n, (_r, _c, _o16) in _LAYOUT16.items():
        _, _, _o = _LAYOUT[_n]
        wc16[:_r, _o16:_o16 + _c] = wconst[:_r, _o:_o + _c].astype(
            ml_dtypes.bfloat16)

    in_maps = []
    for c in range(N_CORES):
        sl = slice(c * BL, (c + 1) * BL)
        in_maps.append({
            "xseq": np.ascontiguousarray(x_seq[sl]),
            "levy": np.ascontiguousarray(levy[:, sl, :].transpose(0, 2, 1)),
            "xlast": np.ascontiguousarray(x_seq[sl, -1, :].T),
            "auxf": np.ascontiguousarray(aux[sl].T),
            "wconst": wconst,
            "wconst16": wc16,
        })

    if _NC_CACHE is None:
        _NC_CACHE = build_nc()
    res = run_bass_kernel_spmd(_NC_CACHE, in_maps, core_ids=list(range(N_CORES)))
    LAST_RESULTS = res

    mean = np.empty((B,), np.float32)
    log_sigma = np.empty((B,), np.float32)
    dir_logit = np.empty((B, 2), np.float32)
    for c in range(N_CORES):
        o = res.results[c]["out4"]           # [4, BL]
        sl = slice(c * BL, (c + 1) * BL)
        mean[sl] = o[0]
        log_sigma[sl] = o[1]
        dir_logit[sl, 0] = o[2]
        dir_logit[sl, 1] = o[3]
    return mean, log_sigma, dir_logit
